# revision 8
# baseline (speedup 1.0000x reference)
# GQA attention kernel for Trainium2, TP-8 over heads.
#
# Sharding: 8 cores, each owns 4 query heads + 1 KV head (tensor parallel).
# Each core computes x @ wq_shard / wk / wv, RoPE, causal flash-style
# attention for its heads, and a partial output projection with its 256
# rows of wo. The host sums the 8 partials (the TP all-reduce).
#
# Layout strategy (contraction dim must sit on SBUF partitions):
#   x^T tiles made on PE (identity transpose) feed Q^T/K^T/V^T projections.
#   Attention runs in the transposed domain: S^T[ki,qi] = K^T.T @ Q^T needs
#   no further transposes; softmax sums come free from a ones column
#   appended to V in the A@V matmul (row 64 of O' = sum_k exp(S)).
#   O^T[hd,qi] is exactly the lhsT the output projection needs.
# All matmuls run as float32r (TF32-like, 1 cycle/row at N>=256).

import numpy as np
from functools import lru_cache

DIM = 2048
HD = 64
B = 2
L = 2048
R = B * L
NCORES = 8
NHC = 4          # q heads per core
QH_COLS = NHC * HD   # 256 wq cols per core
KT = DIM // 128      # 16 k-tiles over the contraction dim
QC = 512             # query chunk (matmul N)
SUB = 256            # phase-A row sub-chunk
ROPE_BASE = 10000.0


def _rope_tables():
    inv_freq = 1.0 / (ROPE_BASE ** (np.arange(0, HD, 2, dtype=np.float64) / HD))
    t = np.arange(L, dtype=np.float64)
    freqs = np.outer(t, inv_freq)            # [L, 32]
    c32 = np.cos(freqs).T.astype(np.float32)  # [32, L]
    s32 = np.sin(freqs).T.astype(np.float32)
    cos128 = np.tile(c32, (4, 1))            # [128, L]
    sinsg = np.tile(np.concatenate([-s32, s32], axis=0), (2, 1))  # [128, L]
    return np.ascontiguousarray(cos128), np.ascontiguousarray(sinsg)


def _masks():
    # m[p, t, f] = 1 if key_pos(128*t + p) <= query_pos(f) within a diagonal
    # 512-wide query chunk; t = ki-tile offset inside the chunk.
    p = np.arange(128)[:, None, None]
    t = np.arange(4)[None, :, None]
    f = np.arange(QC)[None, None, :]
    return (128 * t + p <= f).astype(np.float32)


@lru_cache(maxsize=1)
def _program():
    import concourse.bass as bass
    import concourse.mybir as mybir
    import concourse.tile as tile
    from concourse import bacc
    from contextlib import ExitStack

    f32 = mybir.dt.float32
    f32r = mybir.dt.float32r
    EXP = mybir.ActivationFunctionType.Exp

    def r_(ap):
        return ap.bitcast(f32r)

    nc = bacc.Bacc(None, target_bir_lowering=False)
    x_d = nc.declare_dram_parameter("x", [R, DIM], f32, isOutput=False)
    wq_d = nc.declare_dram_parameter("wq", [DIM, QH_COLS], f32, isOutput=False)
    wkv_d = nc.declare_dram_parameter("wkv", [DIM, 128], f32, isOutput=False)
    wo_d = nc.declare_dram_parameter("wo", [QH_COLS, DIM], f32, isOutput=False)
    cos_d = nc.declare_dram_parameter("cosf", [128, L], f32, isOutput=False)
    sin_d = nc.declare_dram_parameter("sinf", [128, L], f32, isOutput=False)
    msk_d = nc.declare_dram_parameter("masks", [128, 4, QC], f32, isOutput=False)
    idn_d = nc.declare_dram_parameter("idn", [128, 128], f32, isOutput=False)
    out_d = nc.declare_dram_parameter("out", [R, DIM], f32, isOutput=True)

    NSUB = L // SUB           # 8 sub-chunks per batch in phase A
    with tile.TileContext(nc) as tc, ExitStack() as top, \
            nc.allow_low_precision(reason="fp32r matmul pipeline"):
        const = top.enter_context(tc.tile_pool(name="const", bufs=1))
        resid = top.enter_context(tc.tile_pool(name="resid", bufs=1))

        cos_sb = const.tile([128, L], f32)
        sin_sb = const.tile([128, L], f32)
        msk_sb = const.tile([128, 4, QC], f32)
        idn_r = const.tile([128, 128], f32r)
        idn_f = const.tile([64, 64], f32)
        wq_sb = const.tile([128, KT, QH_COLS], f32r)
        wkv_sb = const.tile([128, KT, 128], f32r)
        wo_sb = const.tile([128, 2, DIM], f32r)
        nc.sync.dma_start(out=cos_sb, in_=cos_d[:, :])
        nc.sync.dma_start(out=sin_sb, in_=sin_d[:, :])
        nc.sync.dma_start(out=msk_sb, in_=msk_d[:, :, :])
        nc.sync.dma_start(out=idn_r, in_=idn_d[:, :].bitcast(f32r))
        nc.sync.dma_start(out=idn_f, in_=idn_d[0:64, 0:64])
        ones_f = const.tile([1, 64], f32)
        nc.vector.memset(ones_f, 1.0)
        ones_sb = const.tile([1, 64], f32r)
        nc.vector.tensor_copy(ones_sb[:, :], ones_f[:, :])
        onecol_f = const.tile([128, KT, 1], f32)
        nc.vector.memset(onecol_f, 1.0)
        for k in range(KT):
            nc.sync.dma_start(out=wq_sb[:, k, :],
                              in_=wq_d[k * 128:(k + 1) * 128, :].bitcast(f32r))
            nc.sync.dma_start(out=wkv_sb[:, k, :],
                              in_=wkv_d[k * 128:(k + 1) * 128, :].bitcast(f32r))
        nc.sync.dma_start(out=wo_sb[:, 0, :], in_=wo_d[0:128, :].bitcast(f32r))
        nc.sync.dma_start(out=wo_sb[:, 1, :], in_=wo_d[128:256, :].bitcast(f32r))

        # per-batch resident tiles (tags reused across the two batches)
        for b in range(B):
            qt = [resid.tile([128, L], f32r, tag=f"qt{m}", name=f"qt{b}_{m}") for m in range(2)]
            krep = resid.tile([128, L], f32r, tag="krep", name=f"krep{b}")
            v_sb = resid.tile([128, KT, 65], f32r, tag="v_sb", name=f"v_sb{b}")
            ot = [resid.tile([128, L], f32r, tag=f"ot{m}", name=f"ot{b}_{m}") for m in range(2)]
            nc.vector.tensor_copy(v_sb[:, :, 64:65], onecol_f[:, :, :])

            # ---------------- phase A: x^T, Q^T/K^T/V^T + RoPE ----------
            with ExitStack() as ctx:
                wk = ctx.enter_context(tc.tile_pool(name=f"wkA{b}", bufs=2))
                ps_t = ctx.enter_context(
                    tc.tile_pool(name=f"psT{b}", bufs=3, space="PSUM"))
                ps_p = ctx.enter_context(
                    tc.tile_pool(name=f"psP{b}", bufs=2, space="PSUM"))
                for s in range(NSUB):
                    row0 = b * L + s * SUB
                    ls = slice(s * SUB, (s + 1) * SUB)   # within-batch cols
                    xn = wk.tile([128, SUB // 128, DIM], f32r, tag="xn")
                    for i in range(SUB // 128):
                        nc.sync.dma_start(
                            out=xn[:, i, :],
                            in_=x_d[row0 + i * 128: row0 + (i + 1) * 128,
                                    :].bitcast(f32r))
                    xt = wk.tile([128, KT, SUB], f32r, tag="xt")
                    for k in range(KT):
                        for i in range(SUB // 128):
                            tp = ps_t.tile([128, 128], f32r, tag="tp")
                            nc.tensor.transpose(
                                tp[:, :],
                                xn[:, i, k * 128:(k + 1) * 128],
                                idn_r[:, :])
                            nc.vector.tensor_copy(
                                xt[:, k, i * 128:(i + 1) * 128], tp[:, :])
                    # Q^T (two 128-row groups of head dims)
                    for m in range(2):
                        qps = ps_p.tile([128, SUB], f32, tag="qps")
                        for k in range(KT):
                            nc.tensor.matmul(
                                qps[:, :],
                                wq_sb[:, k, m * 128:(m + 1) * 128],
                                xt[:, k, :],
                                start=(k == 0), stop=(k == KT - 1))
                        q_sb = wk.tile([128, SUB], f32, tag="q_sb")
                        nc.vector.tensor_copy(q_sb[:, :], qps[:, :])
                        qsh = wk.tile([128, SUB], f32, tag="qsh")
                        for lo in (0, 64):
                            nc.sync.dma_start(out=qsh[lo:lo + 32, :],
                                              in_=q_sb[lo + 32:lo + 64, :])
                            nc.sync.dma_start(out=qsh[lo + 32:lo + 64, :],
                                              in_=q_sb[lo:lo + 32, :])
                        t1 = wk.tile([128, SUB], f32, tag="t1")
                        nc.vector.tensor_mul(t1[:, :], q_sb[:, :], cos_sb[:, ls])
                        nc.vector.tensor_mul(qt[m][:, ls], qsh[:, :], sin_sb[:, ls])
                        nc.vector.tensor_add(qt[m][:, ls], qt[m][:, ls], t1[:, :])
                    # K^T | V^T fused projection
                    kvps = ps_p.tile([128, SUB], f32, tag="kvps")
                    for k in range(KT):
                        nc.tensor.matmul(
                            kvps[:, :], wkv_sb[:, k, :], xt[:, k, :],
                            start=(k == 0), stop=(k == KT - 1))
                    k_sb = wk.tile([64, SUB], f32, tag="k_sb")
                    nc.vector.tensor_copy(k_sb[:, :], kvps[0:64, :])
                    ksh = wk.tile([64, SUB], f32, tag="ksh")
                    nc.sync.dma_start(out=ksh[0:32, :], in_=k_sb[32:64, :])
                    nc.sync.dma_start(out=ksh[32:64, :], in_=k_sb[0:32, :])
                    t2 = wk.tile([64, SUB], f32, tag="t2")
                    nc.vector.tensor_mul(t2[:, :], k_sb[:, :], cos_sb[0:64, ls])
                    nc.vector.tensor_mul(krep[0:64, ls], ksh[:, :], sin_sb[0:64, ls])
                    nc.vector.tensor_add(krep[0:64, ls], krep[0:64, ls], t2[:, :])
                    nc.sync.dma_start(out=krep[64:128, ls], in_=krep[0:64, ls])
                    vT = wk.tile([64, SUB], f32, tag="vT")
                    nc.vector.tensor_copy(vT[:, :], kvps[64:128, :])
                    for i in range(SUB // 128):
                        vp = ps_t.tile([128, 64], f32, tag="tp")
                        nc.tensor.transpose(
                            vp[:, :], vT[:, i * 128:(i + 1) * 128],
                            idn_f[:, :])
                        nc.vector.tensor_copy(
                            v_sb[:, s * (SUB // 128) + i, 0:64], vp[:, :])

            # ---------------- attention --------------------------------
            with ExitStack() as ctx:
                wk2 = ctx.enter_context(tc.tile_pool(name=f"wkB{b}", bufs=3))
                nrm = ctx.enter_context(tc.tile_pool(name=f"nrm{b}", bufs=2))
                ps_s = ctx.enter_context(
                    tc.tile_pool(name=f"psS{b}", bufs=2, space="PSUM"))
                ps_o = ctx.enter_context(
                    tc.tile_pool(name=f"psO{b}", bufs=1, space="PSUM"))
                ps_r = ctx.enter_context(
                    tc.tile_pool(name=f"psR{b}", bufs=2, space="PSUM"))
                for m in range(2):
                    for c in range(L // QC):
                        qs = slice(c * QC, (c + 1) * QC)
                        o_ps = [ps_o.tile([65, QC], f32, tag=f"ops{h}", name=f"ops_{h}")
                                for h in range(2)]
                        nkt = 4 * c + 4
                        for g in range(nkt):
                            ks = slice(g * 128, (g + 1) * 128)
                            s_ps = [ps_s.tile([128, QC], f32, tag=f"sps{h}", name=f"sps_{h}")
                                    for h in range(2)]
                            e_sb = [wk2.tile([128, QC], f32r, tag=f"esb{h}", name=f"esb_{h}")
                                    for h in range(2)]
                            for h in range(2):
                                nc.tensor.matmul(
                                    s_ps[h][:, :],
                                    krep[h * 64:(h + 1) * 64, ks],
                                    qt[m][h * 64:(h + 1) * 64, qs],
                                    start=True, stop=True,
                                    tile_position=(h * 64, 0))
                                nc.scalar.activation(
                                    e_sb[h][:, :], s_ps[h][:, :], EXP,
                                    scale=float(1.0 / np.sqrt(HD)))
                                if g >= 4 * c:
                                    nc.vector.tensor_mul(
                                        e_sb[h][:, :], e_sb[h][:, :],
                                        msk_sb[:, g - 4 * c, :])
                                nc.tensor.matmul(
                                    o_ps[h][:, :],
                                    v_sb[:, g, :], e_sb[h][:, :],
                                    start=(g == 0), stop=(g == nkt - 1))
                        for h in range(2):
                            rrec_f = nrm.tile([1, QC], f32, tag="rrec_f")
                            nc.vector.reciprocal(rrec_f[:, :], o_ps[h][64:65, :])
                            rrec = nrm.tile([1, QC], f32r, tag="rrec")
                            nc.vector.tensor_copy(rrec[:, :], rrec_f[:, :])
                            repl = ps_r.tile([64, QC], f32, tag="repl")
                            nc.tensor.matmul(
                                repl[:, :], ones_sb[:, :], rrec[:, :],
                                start=True, stop=True)
                            repl_sb = nrm.tile([64, QC], f32, tag="repl_sb")
                            nc.vector.tensor_copy(repl_sb[:, :], repl[:, :])
                            nc.vector.tensor_mul(
                                ot[m][h * 64:(h + 1) * 64, qs],
                                o_ps[h][0:64, :], repl_sb[:, :])

            # ---------------- output projection (partial) ---------------
            with ExitStack() as ctx:
                st = ctx.enter_context(tc.tile_pool(name=f"st{b}", bufs=3))
                ps_c = ctx.enter_context(
                    tc.tile_pool(name=f"psC{b}", bufs=4, space="PSUM"))
                for rq in range(L // 128):
                    ms = slice(rq * 128, (rq + 1) * 128)
                    stage = st.tile([128, DIM], f32, tag="stage")
                    for ncol in range(DIM // QC):
                        ops = ps_c.tile([128, QC], f32, tag="op")
                        for k2 in range(2):
                            nc.tensor.matmul(
                                ops[:, :],
                                ot[k2][:, ms],
                                wo_sb[:, k2, ncol * QC:(ncol + 1) * QC],
                                start=(k2 == 0), stop=(k2 == 1))
                        nc.vector.tensor_copy(
                            stage[:, ncol * QC:(ncol + 1) * QC], ops[:, :])
                    nc.sync.dma_start(
                        out=out_d[b * L + rq * 128: b * L + (rq + 1) * 128, :],
                        in_=stage[:, :])
    if not nc.is_finalized():
        nc.finalize()
    return nc


def _host_inputs(x, wq, wk, wv, wo):
    xf = np.ascontiguousarray(x.reshape(R, DIM).astype(np.float32))
    cosf, sinf = _rope_tables()
    msk = np.ascontiguousarray(_masks())
    idn = np.eye(128, dtype=np.float32)
    maps = []
    for c in range(NCORES):
        hs = slice(c * QH_COLS, (c + 1) * QH_COLS)
        ks = slice(c * HD, (c + 1) * HD)
        maps.append({
            "x": xf,
            "wq": np.ascontiguousarray(wq[:, hs].astype(np.float32)),
            "wkv": np.ascontiguousarray(
                np.concatenate([wk[:, ks], wv[:, ks]], axis=1).astype(np.float32)),
            "wo": np.ascontiguousarray(wo[hs, :].astype(np.float32)),
            "cosf": cosf, "sinf": sinf, "masks": msk, "idn": idn,
        })
    return maps


def kernel(x, wq, wk, wv, wo):
    from concourse.bass_utils import run_bass_kernel_spmd
    nc = _program()
    maps = _host_inputs(x, wq, wv=wv, wk=wk, wo=wo)
    res = run_bass_kernel_spmd(nc, maps, list(range(NCORES)))
    acc = np.zeros((R, DIM), dtype=np.float64)
    for i in range(NCORES):
        acc += res.results[i]["out"].astype(np.float64)
    return acc.astype(np.float32).reshape(B, L, DIM)
